# revision 23
# baseline (speedup 1.0000x reference)
"""BitLinear (ternary-weight linear with absmax activation quantization) on 8 trn2 cores.

Reference computation:
    scale = max(|x|) / 127            (global over all of x)
    x_int = clip(round(x / scale), -128, 127)       # == round(x/scale), |x/scale| <= 127
    out   = (x_int @ W^T) * weight_scale * scale + bias

Sharding: data-parallel over tokens. Each of the 8 cores gets 1024 of the 8192
tokens (as columns of x^T), the full ternary weight (pre-transposed + cast to
bf16 on host: exact, values in {-1,0,1}), and bias. The global absmax needs one
tiny AllReduce(max) across cores; everything else is core-local and the output
needs no collective.

Device kernel per core (all exact in bf16: |x_int| <= 127 and W ternary are
exactly representable; PSUM accumulates in fp32 and every partial sum is an
integer < 2^24):
  1. stream x^T in 1 MiB chunks, reduce local absmax (DVE)
  2. AllReduce(max) of the per-partition max vector [128] across 8 cores,
     then gpsimd partition_all_reduce to collapse partitions
  3. inv = 127/max; re-stream x^T and quantize with the fp32 magic-number
     trick (x*inv + 1.5*2^23, subtract -> RNE integer, cast bf16), keep
     quantized x^T resident in SBUF (8 MiB)
  4. out[tt,oc] += xq[k,tt].T @ wT[k,oc] over 32 k-tiles into PSUM
     (x-stationary, W streaming, 8 PSUM banks = 8 token tiles per out chunk)
  5. evacuate: ACT scale by ws*max/127, DVE add bias, DMA out

Engine-queue discipline (each queue issues strictly in order, so a DMA
stream whose issue is slot-gated must never sit ahead of something needed
earlier): Sync = x pass-1, x pass-2, outputs. GpSimd = broadcasts, first W
prefetch, collective bounce+trigger, remaining W stream. Scalar = compute only.
"""

import numpy as np
import ml_dtypes

import concourse.bass as bass
import concourse.mybir as mybir
import concourse.tile as tile
from concourse import bacc, bass_isa
from concourse.bass_utils import run_bass_kernel_spmd

P = 128
MAGIC = float(1.5 * 2**23)  # fp32 round-to-nearest-even forcing constant
F32 = mybir.dt.float32
BF16 = mybir.dt.bfloat16


def build_nc(NT=1024, K=4096, O=4096, n_cores=8, ocw=512):
    """Build the per-core SPMD program. NT tokens/core, contraction K, O outputs."""
    KT = K // P
    TT = NT // P
    OC = O // ocw
    act = mybir.ActivationFunctionType
    CH = 2  # k-tiles per x DMA chunk (1 MiB)
    NCH = KT // CH
    W_PRE = 8  # W tiles prefetched before the collective section

    nc = bacc.Bacc(
        "TRN2", target_bir_lowering=False, debug=False, num_devices=n_cores
    )
    xt_d = nc.dram_tensor("xt", [K, NT], F32, kind="ExternalInput")
    # weight tiles pre-arranged on host as [OC, KT, 128, ocw] so each DMA is contiguous
    wt_d = nc.dram_tensor("wt", [OC, KT, P, ocw], BF16, kind="ExternalInput")
    bias_d = nc.dram_tensor("bias", [1, O], F32, kind="ExternalInput")
    ws_d = nc.dram_tensor("wscale", [1, 1], F32, kind="ExternalInput")
    out_d = nc.dram_tensor("out", [NT, O], F32, kind="ExternalOutput")

    with tile.TileContext(nc) as tc:
        with (
            tc.tile_pool(name="constp", bufs=1) as constp,
            tc.tile_pool(name="xqp", bufs=1) as xqp,
            tc.tile_pool(name="xfp", bufs=4) as xfp,
            tc.tile_pool(name="xf2p", bufs=4) as xf2p,
            tc.tile_pool(name="tmpp", bufs=3) as tmpp,
            tc.tile_pool(name="wp", bufs=W_PRE) as wp,
            tc.tile_pool(name="outp", bufs=4) as outp,
            tc.tile_pool(name="psp", bufs=8, space="PSUM") as psp,
            tc.tile_pool(name="dramp", bufs=1, space="DRAM") as dramp,
        ):
            # ---- constants ----
            ws_sb = constp.tile([1, 1], F32, name="ws_sb", tag="ws_sb")
            nc.sync.dma_start(out=ws_sb[:], in_=ws_d[:, :])
            bias_row = constp.tile([1, O], F32, name="bias_row", tag="bias_row")
            nc.sync.dma_start(out=bias_row[:], in_=bias_d[:, :])
            bias_bc = constp.tile([P, O], F32, name="bias_bc", tag="bias_bc")
            nc.gpsimd.partition_broadcast(bias_bc[:], bias_row[:], channels=P)
            ws_bc = constp.tile([P, 1], F32, name="ws_bc", tag="ws_bc")
            nc.gpsimd.partition_broadcast(ws_bc[:], ws_sb[:], channels=P)

            # ---- early W prefetch (gpsimd queue, before the collective ops) ----
            w_tiles = {}
            for kt in range(W_PRE):
                w_sb = wp.tile([P, ocw], BF16, name=f"w_0_{kt}", tag="w")
                nc.gpsimd.dma_start(out=w_sb[:], in_=wt_d[0, kt])
                w_tiles[(0, kt)] = w_sb

            # ---- pass 1: local absmax while streaming x (sync queue) ----
            pmax = constp.tile([P, KT], F32, name="pmax", tag="pmax")
            for c in range(NCH):
                xf = xfp.tile([P, CH, NT], F32, name=f"xf1_{c}", tag="xf")
                nc.sync.dma_start(
                    out=xf[:],
                    in_=xt_d[c * CH * P : (c + 1) * CH * P, :].rearrange(
                        "(c p) f -> p c f", p=P
                    ),
                )
                nc.vector.tensor_reduce(
                    out=pmax[:, c * CH : (c + 1) * CH],
                    in_=xf[:],
                    axis=mybir.AxisListType.X,
                    op=mybir.AluOpType.max,
                    apply_absolute_value=True,
                )
            lmax = constp.tile([P, 1], F32, name="lmax", tag="lmax")
            nc.vector.tensor_reduce(
                out=lmax[:],
                in_=pmax[:],
                axis=mybir.AxisListType.X,
                op=mybir.AluOpType.max,
                apply_absolute_value=True,
            )

            # ---- global max across cores ----
            # cc_in rides the Sync queue right after pass-1: its lmax wait also
            # gates pass-2's issue, keeping the re-read off pass-1's bandwidth.
            cc_in = dramp.tile([P, 1], F32, name="cc_in", tag="cc_in")
            nc.sync.dma_start(out=cc_in[:], in_=lmax[:])
            if n_cores > 1:
                cc_out = dramp.tile([P, 1], F32, name="cc_out", tag="cc_out")
                nc.gpsimd.collective_compute(
                    "AllReduce",
                    mybir.AluOpType.max,
                    replica_groups=[list(range(n_cores))],
                    ins=[cc_in[:].opt()],
                    outs=[cc_out[:].opt()],
                )
                gsrc = cc_out
            else:
                gsrc = cc_in
            gmax_v = constp.tile([P, 1], F32, name="gmax_v", tag="gmax_v")
            nc.gpsimd.dma_start(out=gmax_v[:], in_=gsrc[:])
            gmax = constp.tile([P, 1], F32, name="gmax", tag="gmax")
            nc.gpsimd.partition_all_reduce(
                gmax[:], gmax_v[:], channels=P, reduce_op=bass_isa.ReduceOp.max
            )
            # guard zero, inv = 127/max, csc = ws*max/127
            gmax2 = constp.tile([P, 1], F32, name="gmax2", tag="gmax2")
            nc.vector.tensor_scalar_max(gmax2[:], gmax[:], 1e-30)
            rec = constp.tile([P, 1], F32, name="rec", tag="rec")
            nc.vector.reciprocal(rec[:], gmax2[:])
            inv = constp.tile([P, 1], F32, name="inv", tag="inv")
            nc.vector.tensor_scalar_mul(inv[:], rec[:], 127.0)
            c0 = constp.tile([P, 1], F32, name="c0", tag="c0")
            nc.vector.tensor_tensor(c0[:], gmax2[:], ws_bc[:], op=mybir.AluOpType.mult)
            csc = constp.tile([P, 1], F32, name="csc", tag="csc")
            nc.vector.tensor_scalar_mul(csc[:], c0[:], 1.0 / 127.0)

            # ---- pass 2: re-stream x (sync queue) and quantize ----
            # Work split between ACT and DVE pipelines (DVE pass ~1.8x faster).
            xq_tiles = []
            xf2_list = []
            for c in range(NCH):
                xf2 = xf2p.tile([P, CH, NT], F32, name=f"xf2_{c}", tag="xf2")
                nc.sync.dma_start(
                    out=xf2[:],
                    in_=xt_d[c * CH * P : (c + 1) * CH * P, :].rearrange(
                        "(c p) f -> p c f", p=P
                    ),
                )
                xf2_list.append(xf2)
            for t in range(KT):
                src = xf2_list[t // CH][:, t % CH, :]
                xq = xqp.tile([P, NT], BF16, name=f"xq_{t}", tag=f"xq{t}")
                tmpf = tmpp.tile([P, NT], F32, name=f"tmp_{t}", tag="tmpf")
                if t % 3 == 2:  # ACT-only pipeline (t=0 goes to the faster DVE)
                    nc.scalar.activation(
                        tmpf[:], src, act.Copy, bias=MAGIC, scale=inv[:]
                    )
                    nc.scalar.activation(xq[:], tmpf[:], act.Copy, bias=-MAGIC)
                else:  # DVE-only pipeline
                    nc.vector.tensor_scalar(
                        tmpf[:], src, inv[:], MAGIC,
                        op0=mybir.AluOpType.mult, op1=mybir.AluOpType.add,
                    )
                    nc.vector.tensor_scalar_add(xq[:], tmpf[:], -MAGIC)
                xq_tiles.append(xq)

            # ---- main matmul loop ----
            for oc in range(OC):
                psums = []
                for tt in range(TT):
                    ps = psp.tile([P, ocw], F32, name=f"ps_{oc}_{tt}", tag="ps")
                    psums.append(ps)
                for kt in range(KT):
                    if (oc, kt) in w_tiles:
                        w_sb = w_tiles[(oc, kt)]
                    else:
                        w_sb = wp.tile([P, ocw], BF16, name=f"w_{oc}_{kt}", tag="w")
                        nc.gpsimd.dma_start(out=w_sb[:], in_=wt_d[oc, kt])
                    for tt in range(TT):
                        nc.tensor.matmul(
                            psums[tt][:],
                            xq_tiles[kt][:, tt * P : (tt + 1) * P],
                            w_sb[:],
                            start=(kt == 0),
                            stop=(kt == KT - 1),
                        )
                for tt in range(TT):
                    osb = outp.tile([P, ocw], F32, name=f"o_{oc}_{tt}", tag="o")
                    nc.scalar.activation(
                        osb[:], psums[tt][:], act.Copy, bias=0.0, scale=csc[:]
                    )
                    nc.vector.tensor_tensor(
                        osb[:],
                        osb[:],
                        bias_bc[:, oc * ocw : (oc + 1) * ocw],
                        op=mybir.AluOpType.add,
                    )
                    nc.sync.dma_start(
                        out=out_d[tt * P : (tt + 1) * P, oc * ocw : (oc + 1) * ocw],
                        in_=osb[:],
                    )

    nc.compile()
    return nc


def prep_inputs(x, weight, weight_scale, bias, n_cores=8, ocw=512):
    """Host-side sharding/layout prep. Returns per-core input maps."""
    N, K = x.shape
    O = weight.shape[0]
    NT = N // n_cores
    KT = K // P
    OC = O // ocw

    xt = np.ascontiguousarray(x.T)  # [K, N] f32
    # W^T tile-major: [OC, KT, 128, ocw], each (oc, kt) tile contiguous
    wt = np.ascontiguousarray(weight.T).astype(ml_dtypes.bfloat16)  # [K, O] exact
    wt = np.ascontiguousarray(
        wt.reshape(KT, P, OC, ocw).transpose(2, 0, 1, 3)
    )  # [OC, KT, P, ocw]
    bias_r = np.ascontiguousarray(bias.reshape(1, O), dtype=np.float32)
    ws = np.asarray(weight_scale, dtype=np.float32).reshape(1, 1)

    in_maps = []
    for c in range(n_cores):
        in_maps.append(
            {
                "xt": np.ascontiguousarray(xt[:, c * NT : (c + 1) * NT]),
                "wt": wt,
                "bias": bias_r,
                "wscale": ws,
            }
        )
    return in_maps


_NC_CACHE = {}


def _get_nc(NT, K, O, n_cores, ocw=512):
    key = (NT, K, O, n_cores, ocw)
    if key not in _NC_CACHE:
        _NC_CACHE[key] = build_nc(NT, K, O, n_cores, ocw)
    return _NC_CACHE[key]


def run(x, weight, weight_scale, bias, n_cores=8, trace=False):
    N, K = x.shape
    O = weight.shape[0]
    NT = N // n_cores
    nc = _get_nc(NT, K, O, n_cores)
    in_maps = prep_inputs(x, weight, weight_scale, bias, n_cores=n_cores)
    res = run_bass_kernel_spmd(nc, in_maps, list(range(n_cores)), trace=trace)
    out = np.concatenate([res.results[c]["out"] for c in range(n_cores)], axis=0)
    return out, res


def kernel(x, weight, weight_scale, bias):
    x = np.asarray(x, dtype=np.float32)
    weight = np.asarray(weight)
    bias = np.asarray(bias, dtype=np.float32)
    out, _ = run(x, weight, weight_scale, bias, n_cores=8)
    return out


# revision 25
# speedup vs baseline: 1.0928x; 1.0928x over previous
"""BitLinear (ternary-weight linear with absmax activation quantization) on 8 trn2 cores.

Reference computation:
    scale = max(|x|) / 127            (global over all of x)
    x_int = clip(round(x / scale), -128, 127)       # == round(x/scale), |x/scale| <= 127
    out   = (x_int @ W^T) * weight_scale * scale + bias

Sharding: data-parallel over tokens. Each of the 8 cores gets 1024 of the 8192
tokens (as columns of x^T), the full ternary weight (pre-transposed + cast to
bf16 on host: exact, values in {-1,0,1}), and bias. The global absmax needs one
tiny AllReduce(max) across cores; everything else is core-local and the output
needs no collective.

Device kernel per core (all exact in bf16: |x_int| <= 127 and W ternary are
exactly representable; PSUM accumulates in fp32 and every partial sum is an
integer < 2^24):
  1. stream x^T in 1 MiB chunks, reduce local absmax (DVE)
  2. AllReduce(max) of the per-partition max vector [128] across 8 cores,
     then gpsimd partition_all_reduce to collapse partitions
  3. inv = 127/max; re-stream x^T and quantize with the fp32 magic-number
     trick (x*inv + 1.5*2^23, subtract -> RNE integer, cast bf16), keep
     quantized x^T resident in SBUF (8 MiB)
  4. out[tt,oc] += xq[k,tt].T @ wT[k,oc] over 32 k-tiles into PSUM
     (x-stationary, W streaming, 8 PSUM banks = 8 token tiles per out chunk)
  5. evacuate: ACT scale by ws*max/127, DVE add bias, DMA out

Engine-queue discipline (each queue issues strictly in order, so a DMA
stream whose issue is slot-gated must never sit ahead of something needed
earlier): Sync = x pass-1, x pass-2, outputs. GpSimd = broadcasts, first W
prefetch, collective bounce+trigger, remaining W stream. Scalar = compute only.
"""

import numpy as np
import ml_dtypes

import concourse.bass as bass
import concourse.mybir as mybir
import concourse.tile as tile
from concourse import bacc, bass_isa
from concourse.bass_utils import run_bass_kernel_spmd

P = 128
MAGIC = float(1.5 * 2**23)  # fp32 round-to-nearest-even forcing constant
F32 = mybir.dt.float32
BF16 = mybir.dt.bfloat16


def build_nc(NT=1024, K=4096, O=4096, n_cores=8, ocw=512):
    """Build the per-core SPMD program. NT tokens/core, contraction K, O outputs."""
    KT = K // P
    TT = NT // P
    OC = O // ocw
    act = mybir.ActivationFunctionType
    CH = 2  # k-tiles per x DMA chunk (1 MiB)
    NCH = KT // CH
    W_PRE = min(6, KT)  # W tiles prefetched before the collective section

    nc = bacc.Bacc(
        "TRN2", target_bir_lowering=False, debug=False, num_devices=n_cores
    )
    xt_d = nc.dram_tensor("xt", [K, NT], F32, kind="ExternalInput")
    # weight tiles pre-arranged on host as [OC, KT, 128, ocw] so each DMA is contiguous
    wt_d = nc.dram_tensor("wt", [OC, KT, P, ocw], BF16, kind="ExternalInput")
    bias_d = nc.dram_tensor("bias", [1, O], F32, kind="ExternalInput")
    ws_d = nc.dram_tensor("wscale", [1, 1], F32, kind="ExternalInput")
    out_d = nc.dram_tensor("out", [NT, O], F32, kind="ExternalOutput")

    with tile.TileContext(nc) as tc:
        with (
            tc.tile_pool(name="constp", bufs=1) as constp,
            tc.tile_pool(name="xqp", bufs=1) as xqp,
            tc.tile_pool(name="xfp", bufs=3) as xfp,
            tc.tile_pool(name="xf2p", bufs=4) as xf2p,
            tc.tile_pool(name="tmpp", bufs=3) as tmpp,
            tc.tile_pool(name="wp", bufs=W_PRE) as wp,
            tc.tile_pool(name="outp", bufs=4) as outp,
            tc.tile_pool(name="psp", bufs=8, space="PSUM") as psp,
            tc.tile_pool(name="dramp", bufs=1, space="DRAM") as dramp,
        ):
            # ---- constants ----
            ws_sb = constp.tile([1, 1], F32, name="ws_sb", tag="ws_sb")
            nc.sync.dma_start(out=ws_sb[:], in_=ws_d[:, :])
            bias_row = constp.tile([1, O], F32, name="bias_row", tag="bias_row")
            nc.sync.dma_start(out=bias_row[:], in_=bias_d[:, :])
            bias_bc = constp.tile([P, O], F32, name="bias_bc", tag="bias_bc")
            nc.gpsimd.partition_broadcast(bias_bc[:], bias_row[:], channels=P)
            ws_bc = constp.tile([P, 1], F32, name="ws_bc", tag="ws_bc")
            nc.gpsimd.partition_broadcast(ws_bc[:], ws_sb[:], channels=P)

            # ---- early W prefetch (gpsimd queue, before the collective ops) ----
            w_tiles = {}
            for kt in range(W_PRE):
                w_sb = wp.tile([P, ocw], BF16, name=f"w_0_{kt}", tag="w")
                nc.gpsimd.dma_start(out=w_sb[:], in_=wt_d[0, kt])
                w_tiles[(0, kt)] = w_sb

            # ---- pass 1: local absmax while streaming x (sync queue) ----
            pmax = constp.tile([P, KT], F32, name="pmax", tag="pmax")
            for c in range(NCH):
                xf = xfp.tile([P, CH, NT], F32, name=f"xf1_{c}", tag="xf")
                nc.sync.dma_start(
                    out=xf[:],
                    in_=xt_d[c * CH * P : (c + 1) * CH * P, :].rearrange(
                        "(c p) f -> p c f", p=P
                    ),
                )
                nc.vector.tensor_reduce(
                    out=pmax[:, c * CH : (c + 1) * CH],
                    in_=xf[:],
                    axis=mybir.AxisListType.X,
                    op=mybir.AluOpType.max,
                    apply_absolute_value=True,
                )
            lmax = constp.tile([P, 1], F32, name="lmax", tag="lmax")
            nc.vector.tensor_reduce(
                out=lmax[:],
                in_=pmax[:],
                axis=mybir.AxisListType.X,
                op=mybir.AluOpType.max,
                apply_absolute_value=True,
            )

            # ---- global max across cores ----
            # cc_in rides the Sync queue right after pass-1: its lmax wait also
            # gates pass-2's issue, keeping the re-read off pass-1's bandwidth.
            cc_in = dramp.tile([P, 1], F32, name="cc_in", tag="cc_in")
            nc.sync.dma_start(out=cc_in[:], in_=lmax[:])
            if n_cores > 1:
                cc_out = dramp.tile([P, 1], F32, name="cc_out", tag="cc_out")
                nc.gpsimd.collective_compute(
                    "AllReduce",
                    mybir.AluOpType.max,
                    replica_groups=[list(range(n_cores))],
                    ins=[cc_in[:].opt()],
                    outs=[cc_out[:].opt()],
                )
                gsrc = cc_out
            else:
                gsrc = cc_in
            gmax_v = constp.tile([P, 1], F32, name="gmax_v", tag="gmax_v")
            nc.gpsimd.dma_start(out=gmax_v[:], in_=gsrc[:])
            gmax = constp.tile([P, 1], F32, name="gmax", tag="gmax")
            nc.gpsimd.partition_all_reduce(
                gmax[:], gmax_v[:], channels=P, reduce_op=bass_isa.ReduceOp.max
            )
            # guard zero, inv = 127/max, csc = ws*max/127
            gmax2 = constp.tile([P, 1], F32, name="gmax2", tag="gmax2")
            nc.vector.tensor_scalar_max(gmax2[:], gmax[:], 1e-30)
            rec = constp.tile([P, 1], F32, name="rec", tag="rec")
            nc.vector.reciprocal(rec[:], gmax2[:])
            inv = constp.tile([P, 1], F32, name="inv", tag="inv")
            nc.vector.tensor_scalar_mul(inv[:], rec[:], 127.0)
            c0 = constp.tile([P, 1], F32, name="c0", tag="c0")
            nc.vector.tensor_tensor(c0[:], gmax2[:], ws_bc[:], op=mybir.AluOpType.mult)
            csc = constp.tile([P, 1], F32, name="csc", tag="csc")
            nc.vector.tensor_scalar_mul(csc[:], c0[:], 1.0 / 127.0)

            # ---- pass 2: re-stream x (sync queue) and quantize ----
            # Work split between ACT and DVE pipelines (DVE pass ~1.8x faster).
            xq_tiles = []
            xf2_list = []
            for c in range(NCH):
                xf2 = xf2p.tile([P, CH, NT], F32, name=f"xf2_{c}", tag="xf2")
                nc.sync.dma_start(
                    out=xf2[:],
                    in_=xt_d[c * CH * P : (c + 1) * CH * P, :].rearrange(
                        "(c p) f -> p c f", p=P
                    ),
                )
                xf2_list.append(xf2)
            for t in range(KT):
                src = xf2_list[t // CH][:, t % CH, :]
                xq = xqp.tile([P, NT], BF16, name=f"xq_{t}", tag=f"xq{t}")
                tmpf = tmpp.tile([P, NT], F32, name=f"tmp_{t}", tag="tmpf")
                if t % 3 == 2:  # ACT-only pipeline (t=0 goes to the faster DVE)
                    nc.scalar.activation(
                        tmpf[:], src, act.Copy, bias=MAGIC, scale=inv[:]
                    )
                    nc.scalar.activation(xq[:], tmpf[:], act.Copy, bias=-MAGIC)
                else:  # DVE-only pipeline
                    nc.vector.tensor_scalar(
                        tmpf[:], src, inv[:], MAGIC,
                        op0=mybir.AluOpType.mult, op1=mybir.AluOpType.add,
                    )
                    nc.vector.tensor_scalar_add(xq[:], tmpf[:], -MAGIC)
                xq_tiles.append(xq)

            # ---- main matmul loop ----
            for oc in range(OC):
                psums = []
                for tt in range(TT):
                    ps = psp.tile([P, ocw], F32, name=f"ps_{oc}_{tt}", tag="ps")
                    psums.append(ps)
                for kt in range(KT):
                    if (oc, kt) in w_tiles:
                        w_sb = w_tiles[(oc, kt)]
                    else:
                        w_sb = wp.tile([P, ocw], BF16, name=f"w_{oc}_{kt}", tag="w")
                        nc.gpsimd.dma_start(out=w_sb[:], in_=wt_d[oc, kt])
                    for tt in range(TT):
                        nc.tensor.matmul(
                            psums[tt][:],
                            xq_tiles[kt][:, tt * P : (tt + 1) * P],
                            w_sb[:],
                            start=(kt == 0),
                            stop=(kt == KT - 1),
                        )
                for tt in range(TT):
                    osb = outp.tile([P, ocw], F32, name=f"o_{oc}_{tt}", tag="o")
                    nc.scalar.activation(
                        osb[:], psums[tt][:], act.Copy, bias=0.0, scale=csc[:]
                    )
                    nc.vector.tensor_tensor(
                        osb[:],
                        osb[:],
                        bias_bc[:, oc * ocw : (oc + 1) * ocw],
                        op=mybir.AluOpType.add,
                    )
                    nc.sync.dma_start(
                        out=out_d[tt * P : (tt + 1) * P, oc * ocw : (oc + 1) * ocw],
                        in_=osb[:],
                    )

    nc.compile()
    return nc


def prep_inputs(x, weight, weight_scale, bias, n_cores=8, ocw=512):
    """Host-side sharding/layout prep. Returns per-core input maps."""
    N, K = x.shape
    O = weight.shape[0]
    NT = N // n_cores
    KT = K // P
    OC = O // ocw

    xt = np.ascontiguousarray(x.T)  # [K, N] f32
    # W^T tile-major: [OC, KT, 128, ocw], each (oc, kt) tile contiguous
    wt = np.ascontiguousarray(weight.T).astype(ml_dtypes.bfloat16)  # [K, O] exact
    wt = np.ascontiguousarray(
        wt.reshape(KT, P, OC, ocw).transpose(2, 0, 1, 3)
    )  # [OC, KT, P, ocw]
    bias_r = np.ascontiguousarray(bias.reshape(1, O), dtype=np.float32)
    ws = np.asarray(weight_scale, dtype=np.float32).reshape(1, 1)

    in_maps = []
    for c in range(n_cores):
        in_maps.append(
            {
                "xt": np.ascontiguousarray(xt[:, c * NT : (c + 1) * NT]),
                "wt": wt,
                "bias": bias_r,
                "wscale": ws,
            }
        )
    return in_maps


_NC_CACHE = {}


def _get_nc(NT, K, O, n_cores, ocw=512):
    key = (NT, K, O, n_cores, ocw)
    if key not in _NC_CACHE:
        _NC_CACHE[key] = build_nc(NT, K, O, n_cores, ocw)
    return _NC_CACHE[key]


def run(x, weight, weight_scale, bias, n_cores=8, trace=False):
    N, K = x.shape
    O = weight.shape[0]
    NT = N // n_cores
    nc = _get_nc(NT, K, O, n_cores)
    in_maps = prep_inputs(x, weight, weight_scale, bias, n_cores=n_cores)
    res = run_bass_kernel_spmd(nc, in_maps, list(range(n_cores)), trace=trace)
    out = np.concatenate([res.results[c]["out"] for c in range(n_cores)], axis=0)
    return out, res


def kernel(x, weight, weight_scale, bias):
    x = np.asarray(x, dtype=np.float32)
    weight = np.asarray(weight)
    bias = np.asarray(bias, dtype=np.float32)
    out, _ = run(x, weight, weight_scale, bias, n_cores=8)
    return out
